# revision 15
# baseline (speedup 1.0000x reference)
"""Trainium2 Bass kernel for nn_AutoTransformer_27230092656858 (moe_routing).

Math (per the reference):
  h_k    = relu(x @ W1[k] + b1[k])                      for k in 0..3
  flat   = concat_k( where(readout_x==k, h_k @ W2_k + b2_k, 0) )
  out[readout_t - min_t, b] = flat                      (collision-free scatter)

Strategy: data-parallel over batch (32 cols -> 4 per NeuronCore, 8 cores).
Each core routes its 2048 tokens by readout type (MoE style): gather the
x rows of each type with indirect DMA, transpose on the PE to get X^T,
run both decoder layers as fp32r matmuls for only the owning head, and
indirect-scatter each head's D_k-wide logits into the (pre-zeroed)
per-chunk output tensors.  Routing tables are tiny int32 tensors
computed on the host from readout_x / readout_t.
"""

import sys

if "/opt/trn_rl_repo" not in sys.path:
    sys.path.insert(0, "/opt/trn_rl_repo")

import numpy as np

import concourse.bass as bass
import concourse.mybir as mybir
import concourse.tile as tile
from concourse import bacc
from concourse.bass_utils import run_bass_kernel_spmd
from concourse.masks import make_identity

# Problem shapes (hardcoded per spec)
S, B, C = 512, 32, 1024
HEAD_DIMS = (2048, 2048, 1024, 512)
K = 4
A = sum(HEAD_DIMS)  # 5632
NCORES = 8
BC = B // NCORES  # 4 batch columns per core
NTOK = S * BC  # 2048 tokens per core

F32 = mybir.dt.float32
F32R = mybir.dt.float32r
I32 = mybir.dt.int32
RELU = mybir.ActivationFunctionType.Relu

OOB_SENTINEL = 1 << 20

# Output column blocks: one DRAM tensor per (head, <=1024-wide d-chunk) so
# the indirect scatters never alias and are not serialized against each other.
OUT_BLOCKS = [
    (k, d0, min(1024, HEAD_DIMS[k] - d0))
    for k in range(K)
    for d0 in range(0, HEAD_DIMS[k], 1024)
]

DEFAULT_CFG = dict(
    w1_bufs=1,
    w2_bufs=2,
    g_bufs=6,
    xt_bufs=1,
    ht_bufs=2,
    so_bufs=4,
    tr_psum_bufs=2,
    l1_psum_bufs=2,
    l2_psum_bufs=3,
    w1_split=8,  # number of DMA pieces for each W1[k] (along c_out)
    hoist_transposes=False,  # emit transposes k+1 before L2 k
)

_program_cache: dict = {}


def _build_program(caps, use_b1, use_b2, cfg=None):
    """Build + compile the (shared, SPMD) Bass program.

    caps[k]: token capacity (multiple of 128) for head k, shared by all cores.
    """
    cfg = {**DEFAULT_CFG, **(cfg or {})}
    nc = bacc.Bacc("TRN2", target_bir_lowering=False, debug=False)

    x = nc.dram_tensor("x", [NTOK, C], F32, kind="ExternalInput")
    w1 = nc.dram_tensor("w1", [K, C, C], F32R, kind="ExternalInput")
    b1 = nc.dram_tensor("b1", [K, C], F32, kind="ExternalInput")
    w2 = [
        nc.dram_tensor(f"w2_{k}", [C, HEAD_DIMS[k]], F32R, kind="ExternalInput")
        for k in range(K)
    ]
    b2 = [
        nc.dram_tensor(f"b2_{k}", [HEAD_DIMS[k]], F32R, kind="ExternalInput")
        for k in range(K)
    ]
    njs = [caps[k] // 128 for k in range(K)]
    G = sum(njs)  # total 128-wide index columns per table
    idx = nc.dram_tensor("idx", [128, 2 * G], I32, kind="ExternalInput")
    outs = {}
    for k, d0, wt in OUT_BLOCKS:
        outs[(k, d0)] = nc.dram_tensor(
            f"out_{k}_{d0}", [NTOK, wt], F32, kind="ExternalOutput"
        )

    with tile.TileContext(nc) as tc:
        with (
            tc.tile_pool(name="const", bufs=1) as cpool,
            tc.tile_pool(name="w1p", bufs=cfg["w1_bufs"]) as w1pool,
            tc.tile_pool(name="w2p", bufs=cfg["w2_bufs"]) as w2pool,
            tc.tile_pool(name="gp", bufs=cfg["g_bufs"]) as gpool,
            tc.tile_pool(name="xtp", bufs=cfg["xt_bufs"]) as xtpool,
            tc.tile_pool(name="htp", bufs=cfg["ht_bufs"]) as htpool,
            tc.tile_pool(name="sop", bufs=cfg["so_bufs"]) as sopool,
            tc.tile_pool(name="bp", bufs=1) as bpool,
            tc.tile_pool(
                name="trps", bufs=cfg["tr_psum_bufs"], space="PSUM"
            ) as trpsum,
            tc.tile_pool(
                name="l1ps", bufs=cfg["l1_psum_bufs"], space="PSUM"
            ) as l1psum,
            tc.tile_pool(
                name="l2ps", bufs=cfg["l2_psum_bufs"], space="PSUM"
            ) as l2psum,
        ):
            idx_sb = cpool.tile([128, 2 * G], I32)
            nc.sync.dma_start(idx_sb[:], idx[:])
            ident = cpool.tile([128, 128], F32)
            make_identity(nc, ident[:])
            if use_b2:
                ones1 = cpool.tile([1, 128], F32R)
                nc.gpsimd.memset(ones1[:], 1.0)

            colof = [0]
            for k in range(K):
                colof.append(colof[-1] + njs[k])

            def gather_type(k):
                """Indirect gathers (128 rows each) for head k's tokens."""
                gs = []
                for j in range(njs[k]):
                    g = gpool.tile([128, C], F32, tag="g")
                    nc.gpsimd.indirect_dma_start(
                        out=g[:],
                        out_offset=None,
                        in_=x[:],
                        in_offset=bass.IndirectOffsetOnAxis(
                            ap=idx_sb[:, colof[k] + j : colof[k] + j + 1], axis=0
                        ),
                    )
                    gs.append(g)
                return gs

            def transpose_type(k, gs):
                """PE-transpose gathered rows into X^T [128, 8, capk]."""
                capk = caps[k]
                xt = xtpool.tile([128, 8, capk], F32R, tag="xt")
                for j, g in enumerate(gs):
                    for ci in range(8):
                        pt = trpsum.tile([128, 128], F32, tag="tr")
                        nc.tensor.transpose(
                            pt[:], g[:, ci * 128 : (ci + 1) * 128], ident[:]
                        )
                        nc.vector.tensor_copy(
                            xt[:, ci, j * 128 : (j + 1) * 128], pt[:]
                        )
                return xt

            def load_w1(k):
                # Split along c_out (m) so layer 1's m-th matmul group only
                # depends on its own 512KB slice, not the whole 4MB load.
                w1t = w1pool.tile([128, 8, C], F32R, tag="w1")
                w1r = w1[k].rearrange("(ci p) co -> p ci co", p=128)
                step = C // cfg["w1_split"]
                for i in range(0, C, step):
                    nc.sync.dma_start(
                        w1t[:, :, i : i + step], w1r[:, :, i : i + step]
                    )
                return w1t

            def load_biases(k):
                b1t = b2t = None
                if use_b1:
                    b1t = bpool.tile([128, 8], F32, tag="b1")
                    nc.sync.dma_start(
                        b1t[:], b1[k].rearrange("(o p) -> p o", p=128)
                    )
                if use_b2:
                    b2t = bpool.tile([1, max(HEAD_DIMS)], F32R, tag="b2")
                    nc.sync.dma_start(b2t[:1, : HEAD_DIMS[k]], b2[k][None, :])
                return b1t, b2t

            def layer1(k, w1t, xt, b1t):
                capk = caps[k]
                ht = htpool.tile([128, 8, capk], F32R, tag="ht")
                nch = -(-capk // 512)
                    # balanced chunk sizes (multiples of 64, sum = capk) so no
                    # chunk is so narrow that LDWEIGHTS dominates
                bsz = capk // nch // 64 * 64
                sizes = [bsz] * nch
                sizes[-1] = capk - bsz * (nch - 1)
                starts = [sum(sizes[:i]) for i in range(nch)]
                for m in range(8):
                    for n0, nt in zip(starts, sizes):
                        ps = l1psum.tile([128, 512], F32, tag="l1")
                        for ci in range(8):
                            nc.tensor.matmul(
                                ps[:, :nt],
                                w1t[:, ci, m * 128 : (m + 1) * 128],
                                xt[:, ci, n0 : n0 + nt],
                                start=(ci == 0),
                                stop=(ci == 7),
                            )
                        if use_b1:
                            nc.scalar.activation(
                                ht[:, m, n0 : n0 + nt],
                                ps[:, :nt],
                                RELU,
                                bias=b1t[:, m : m + 1],
                            )
                        else:
                            nc.scalar.activation(
                                ht[:, m, n0 : n0 + nt], ps[:, :nt], RELU
                            )
                return ht

            def layer2(k, ht, b2t):
                nj = njs[k]
                D = HEAD_DIMS[k]
                w2r = w2[k].rearrange("(m p) d -> p m d", p=128)
                for d0 in range(0, D, 1024):
                    wt = min(1024, D - d0)
                    w2c = w2pool.tile([128, 8, 1024], F32R, tag="w2")
                    nc.sync.dma_start(w2c[:, :, :wt], w2r[:, :, d0 : d0 + wt])
                    for j in range(nj):
                        so = sopool.tile([128, 1024], F32, tag="so")
                        for dh in range(0, wt, 512):
                            dt_ = min(512, wt - dh)
                            ps2 = l2psum.tile([128, 512], F32, tag="l2")
                            for m in range(8):
                                nc.tensor.matmul(
                                    ps2[:, :dt_],
                                    ht[:, m, j * 128 : (j + 1) * 128],
                                    w2c[:, m, dh : dh + dt_],
                                    start=(m == 0),
                                    stop=(m == 7 and not use_b2),
                                )
                            if use_b2:
                                nc.tensor.matmul(
                                    ps2[:, :dt_],
                                    ones1[:1, :],
                                    b2t[:1, d0 + dh : d0 + dh + dt_],
                                    start=False,
                                    stop=True,
                                )
                            nc.vector.tensor_copy(
                                so[:, dh : dh + dt_], ps2[:, :dt_]
                            )
                        nc.gpsimd.indirect_dma_start(
                            out=outs[(k, d0)][:],
                            out_offset=bass.IndirectOffsetOnAxis(
                                ap=idx_sb[
                                    :, G + colof[k] + j : G + colof[k] + j + 1
                                ],
                                axis=0,
                            ),
                            in_=so[:, :wt],
                            in_offset=None,
                            bounds_check=NTOK - 1,
                            oob_is_err=False,
                        )

            g_cur = gather_type(0)
            xts = {}
            for k in range(K):
                w1t = load_w1(k)
                b1t, b2t = load_biases(k)
                if k in xts:
                    xt = xts.pop(k)
                else:
                    xt = transpose_type(k, g_cur)
                ht = layer1(k, w1t, xt, b1t)
                if k + 1 < K:
                    g_cur = gather_type(k + 1)
                    if cfg["hoist_transposes"]:
                        xts[k + 1] = transpose_type(k + 1, g_cur)
                layer2(k, ht, b2t)

    nc.compile()
    return nc


def _routing(rx_shard, rt_shard, min_t):
    """Per-core routing tables.

    Returns (counts[k], token_lists[k], target_rows) where token_lists[k]
    holds flat token ids (s*BC + b) of head k in order, and target_rows[t]
    is the output row for flat token t.
    """
    rx_flat = rx_shard.reshape(-1)  # [NTOK], token t = s*BC + b
    ri = rt_shard - min_t[None, :]  # [S, BC]
    b_ids = np.broadcast_to(np.arange(BC, dtype=np.int64)[None, :], ri.shape)
    target = (ri.astype(np.int64) * BC + b_ids).reshape(-1)  # [NTOK]
    lists = [np.nonzero(rx_flat == k)[0] for k in range(K)]
    counts = [len(l) for l in lists]
    return counts, lists, target


def _pack_idx(caps, lists_per_core, targets_per_core):
    """Build the [128, 2G] int32 index tensor for one core."""
    G = sum(c // 128 for c in caps)
    arr = np.zeros((128, 2 * G), dtype=np.int32)
    col = 0
    for k in range(K):
        capk = caps[k]
        nj = capk // 128
        lst = lists_per_core[k]
        g = np.zeros(capk, dtype=np.int32)  # gather pad -> row 0 (safe)
        g[: len(lst)] = lst
        s = np.full(capk, OOB_SENTINEL, dtype=np.int32)  # scatter pad -> skipped
        s[: len(lst)] = targets_per_core[lst]
        for j in range(nj):
            arr[:, col + j] = g[j * 128 : (j + 1) * 128]
            arr[:, G + col + j] = s[j * 128 : (j + 1) * 128]
        col += nj
    return arr


def _prepare(inputs, cfg=None):
    """Shared host-side prep for kernel() and profiling runs."""
    x = np.ascontiguousarray(np.asarray(inputs["x"], dtype=np.float32))
    rx = np.asarray(inputs["readout_x"], dtype=np.int64)
    rt = np.asarray(inputs["readout_t"], dtype=np.int64)
    W1 = np.ascontiguousarray(np.asarray(inputs["W1"], dtype=np.float32))
    b1 = np.ascontiguousarray(np.asarray(inputs["b1"], dtype=np.float32))
    W2 = [
        np.ascontiguousarray(np.asarray(inputs[f"W2_{k}"], dtype=np.float32))
        for k in range(K)
    ]
    b2 = [
        np.ascontiguousarray(np.asarray(inputs[f"b2_{k}"], dtype=np.float32))
        for k in range(K)
    ]
    min_t = rt.min(axis=0)  # [B]

    per_core = []
    for c in range(NCORES):
        bsl = slice(c * BC, (c + 1) * BC)
        counts, lists, target = _routing(rx[:, bsl], rt[:, bsl], min_t[bsl])
        per_core.append((counts, lists, target))

    caps = tuple(
        max(128, int(-(-max(pc[0][k] for pc in per_core) // 128)) * 128)
        for k in range(K)
    )
    use_b1 = bool(np.any(b1))
    use_b2 = bool(np.any(np.concatenate([v.ravel() for v in b2])))

    key = (caps, use_b1, use_b2, tuple(sorted((cfg or {}).items())))
    if key not in _program_cache:
        _program_cache[key] = _build_program(caps, use_b1, use_b2, cfg)
    nc = _program_cache[key]

    in_maps = []
    for c in range(NCORES):
        counts, lists, target = per_core[c]
        x_shard = np.ascontiguousarray(
            x[:, c * BC : (c + 1) * BC, :]
        ).reshape(NTOK, C)
        m = {
            "x": x_shard,
            "w1": W1,
            "b1": b1,
            "idx": _pack_idx(caps, lists, target),
        }
        for k in range(K):
            m[f"w2_{k}"] = W2[k]
            m[f"b2_{k}"] = b2[k]
        in_maps.append(m)
    return nc, in_maps


def _run(inputs, cfg=None, **run_kwargs):
    nc, in_maps = _prepare(inputs, cfg)
    res = run_bass_kernel_spmd(
        nc, in_maps, core_ids=list(range(NCORES)), **run_kwargs
    )
    shards = []
    for c in range(NCORES):
        pieces = [res.results[c][f"out_{k}_{d0}"] for k, d0, _ in OUT_BLOCKS]
        shards.append(np.concatenate(pieces, axis=-1).reshape(S, BC, A))
    full = np.concatenate(shards, axis=1)
    return full, res


def kernel(**inputs) -> np.ndarray:
    full, _ = _run(inputs)
    return full


# revision 16
# speedup vs baseline: 1.0410x; 1.0410x over previous
"""Trainium2 Bass kernel for nn_AutoTransformer_27230092656858 (moe_routing).

Math (per the reference):
  h_k    = relu(x @ W1[k] + b1[k])                      for k in 0..3
  flat   = concat_k( where(readout_x==k, h_k @ W2_k + b2_k, 0) )
  out[readout_t - min_t, b] = flat                      (collision-free scatter)

Strategy: data-parallel over batch (32 cols -> 4 per NeuronCore, 8 cores).
Each core routes its 2048 tokens by readout type (MoE style): gather the
x rows of each type with indirect DMA, transpose on the PE to get X^T,
run both decoder layers as fp32r matmuls for only the owning head, and
indirect-scatter each head's D_k-wide logits into the (pre-zeroed)
per-chunk output tensors.  Routing tables are tiny int32 tensors
computed on the host from readout_x / readout_t.
"""

import sys

if "/opt/trn_rl_repo" not in sys.path:
    sys.path.insert(0, "/opt/trn_rl_repo")

import numpy as np

import concourse.bass as bass
import concourse.mybir as mybir
import concourse.tile as tile
from concourse import bacc
from concourse.bass_utils import run_bass_kernel_spmd
from concourse.masks import make_identity

# Problem shapes (hardcoded per spec)
S, B, C = 512, 32, 1024
HEAD_DIMS = (2048, 2048, 1024, 512)
K = 4
A = sum(HEAD_DIMS)  # 5632
NCORES = 8
BC = B // NCORES  # 4 batch columns per core
NTOK = S * BC  # 2048 tokens per core

F32 = mybir.dt.float32
F32R = mybir.dt.float32r
I32 = mybir.dt.int32
RELU = mybir.ActivationFunctionType.Relu

OOB_SENTINEL = 1 << 20

# Output column blocks: one DRAM tensor per (head, <=1024-wide d-chunk) so
# the indirect scatters never alias and are not serialized against each other.
OUT_BLOCKS = [
    (k, d0, min(1024, HEAD_DIMS[k] - d0))
    for k in range(K)
    for d0 in range(0, HEAD_DIMS[k], 1024)
]

DEFAULT_CFG = dict(
    w1_bufs=1,
    w2_bufs=2,
    g_bufs=6,
    xt_bufs=1,
    ht_bufs=2,
    so_bufs=4,
    tr_psum_bufs=2,
    l1_psum_bufs=2,
    l2_psum_bufs=3,
    w1_split=2,  # number of DMA pieces for each W1[k] (along c_out)
    hoist_transposes=False,  # emit transposes k+1 before L2 k
)

_program_cache: dict = {}


def _build_program(caps, use_b1, use_b2, cfg=None):
    """Build + compile the (shared, SPMD) Bass program.

    caps[k]: token capacity (multiple of 128) for head k, shared by all cores.
    """
    cfg = {**DEFAULT_CFG, **(cfg or {})}
    nc = bacc.Bacc("TRN2", target_bir_lowering=False, debug=False)

    x = nc.dram_tensor("x", [NTOK, C], F32, kind="ExternalInput")
    w1 = nc.dram_tensor("w1", [K, C, C], F32R, kind="ExternalInput")
    b1 = nc.dram_tensor("b1", [K, C], F32, kind="ExternalInput")
    w2 = [
        nc.dram_tensor(f"w2_{k}", [C, HEAD_DIMS[k]], F32R, kind="ExternalInput")
        for k in range(K)
    ]
    b2 = [
        nc.dram_tensor(f"b2_{k}", [HEAD_DIMS[k]], F32R, kind="ExternalInput")
        for k in range(K)
    ]
    njs = [caps[k] // 128 for k in range(K)]
    G = sum(njs)  # total 128-wide index columns per table
    idx = nc.dram_tensor("idx", [128, 2 * G], I32, kind="ExternalInput")
    outs = {}
    for k, d0, wt in OUT_BLOCKS:
        outs[(k, d0)] = nc.dram_tensor(
            f"out_{k}_{d0}", [NTOK, wt], F32, kind="ExternalOutput"
        )

    with tile.TileContext(nc) as tc:
        with (
            tc.tile_pool(name="const", bufs=1) as cpool,
            tc.tile_pool(name="w1p", bufs=cfg["w1_bufs"]) as w1pool,
            tc.tile_pool(name="w2p", bufs=cfg["w2_bufs"]) as w2pool,
            tc.tile_pool(name="gp", bufs=cfg["g_bufs"]) as gpool,
            tc.tile_pool(name="xtp", bufs=cfg["xt_bufs"]) as xtpool,
            tc.tile_pool(name="htp", bufs=cfg["ht_bufs"]) as htpool,
            tc.tile_pool(name="sop", bufs=cfg["so_bufs"]) as sopool,
            tc.tile_pool(name="bp", bufs=1) as bpool,
            tc.tile_pool(
                name="trps", bufs=cfg["tr_psum_bufs"], space="PSUM"
            ) as trpsum,
            tc.tile_pool(
                name="l1ps", bufs=cfg["l1_psum_bufs"], space="PSUM"
            ) as l1psum,
            tc.tile_pool(
                name="l2ps", bufs=cfg["l2_psum_bufs"], space="PSUM"
            ) as l2psum,
        ):
            idx_sb = cpool.tile([128, 2 * G], I32)
            nc.sync.dma_start(idx_sb[:], idx[:])
            ident = cpool.tile([128, 128], F32)
            make_identity(nc, ident[:])
            if use_b2:
                ones1 = cpool.tile([1, 128], F32R)
                nc.gpsimd.memset(ones1[:], 1.0)

            colof = [0]
            for k in range(K):
                colof.append(colof[-1] + njs[k])

            def gather_type(k):
                """Indirect gathers (128 rows each) for head k's tokens."""
                gs = []
                for j in range(njs[k]):
                    g = gpool.tile([128, C], F32, tag="g")
                    nc.gpsimd.indirect_dma_start(
                        out=g[:],
                        out_offset=None,
                        in_=x[:],
                        in_offset=bass.IndirectOffsetOnAxis(
                            ap=idx_sb[:, colof[k] + j : colof[k] + j + 1], axis=0
                        ),
                    )
                    gs.append(g)
                return gs

            def transpose_type(k, gs):
                """PE-transpose gathered rows into X^T [128, 8, capk]."""
                capk = caps[k]
                xt = xtpool.tile([128, 8, capk], F32R, tag="xt")
                for j, g in enumerate(gs):
                    for ci in range(8):
                        pt = trpsum.tile([128, 128], F32, tag="tr")
                        nc.tensor.transpose(
                            pt[:], g[:, ci * 128 : (ci + 1) * 128], ident[:]
                        )
                        nc.vector.tensor_copy(
                            xt[:, ci, j * 128 : (j + 1) * 128], pt[:]
                        )
                return xt

            def load_w1(k):
                # Split along c_out (m) so layer 1's m-th matmul group only
                # depends on its own 512KB slice, not the whole 4MB load.
                w1t = w1pool.tile([128, 8, C], F32R, tag="w1")
                w1r = w1[k].rearrange("(ci p) co -> p ci co", p=128)
                step = C // cfg["w1_split"]
                for i in range(0, C, step):
                    nc.sync.dma_start(
                        w1t[:, :, i : i + step], w1r[:, :, i : i + step]
                    )
                return w1t

            def load_biases(k):
                b1t = b2t = None
                if use_b1:
                    b1t = bpool.tile([128, 8], F32, tag="b1")
                    nc.sync.dma_start(
                        b1t[:], b1[k].rearrange("(o p) -> p o", p=128)
                    )
                if use_b2:
                    b2t = bpool.tile([1, max(HEAD_DIMS)], F32R, tag="b2")
                    nc.sync.dma_start(b2t[:1, : HEAD_DIMS[k]], b2[k][None, :])
                return b1t, b2t

            def layer1(k, w1t, xt, b1t):
                capk = caps[k]
                ht = htpool.tile([128, 8, capk], F32R, tag="ht")
                nch = -(-capk // 512)
                    # balanced chunk sizes (multiples of 64, sum = capk) so no
                    # chunk is so narrow that LDWEIGHTS dominates
                bsz = capk // nch // 64 * 64
                sizes = [bsz] * nch
                sizes[-1] = capk - bsz * (nch - 1)
                starts = [sum(sizes[:i]) for i in range(nch)]
                for m in range(8):
                    for n0, nt in zip(starts, sizes):
                        ps = l1psum.tile([128, 512], F32, tag="l1")
                        for ci in range(8):
                            nc.tensor.matmul(
                                ps[:, :nt],
                                w1t[:, ci, m * 128 : (m + 1) * 128],
                                xt[:, ci, n0 : n0 + nt],
                                start=(ci == 0),
                                stop=(ci == 7),
                            )
                        if use_b1:
                            nc.scalar.activation(
                                ht[:, m, n0 : n0 + nt],
                                ps[:, :nt],
                                RELU,
                                bias=b1t[:, m : m + 1],
                            )
                        else:
                            nc.scalar.activation(
                                ht[:, m, n0 : n0 + nt], ps[:, :nt], RELU
                            )
                return ht

            def layer2(k, ht, b2t):
                nj = njs[k]
                D = HEAD_DIMS[k]
                w2r = w2[k].rearrange("(m p) d -> p m d", p=128)
                for d0 in range(0, D, 1024):
                    wt = min(1024, D - d0)
                    w2c = w2pool.tile([128, 8, 1024], F32R, tag="w2")
                    nc.sync.dma_start(w2c[:, :, :wt], w2r[:, :, d0 : d0 + wt])
                    for j in range(nj):
                        so = sopool.tile([128, 1024], F32, tag="so")
                        for dh in range(0, wt, 512):
                            dt_ = min(512, wt - dh)
                            ps2 = l2psum.tile([128, 512], F32, tag="l2")
                            for m in range(8):
                                nc.tensor.matmul(
                                    ps2[:, :dt_],
                                    ht[:, m, j * 128 : (j + 1) * 128],
                                    w2c[:, m, dh : dh + dt_],
                                    start=(m == 0),
                                    stop=(m == 7 and not use_b2),
                                )
                            if use_b2:
                                nc.tensor.matmul(
                                    ps2[:, :dt_],
                                    ones1[:1, :],
                                    b2t[:1, d0 + dh : d0 + dh + dt_],
                                    start=False,
                                    stop=True,
                                )
                            nc.vector.tensor_copy(
                                so[:, dh : dh + dt_], ps2[:, :dt_]
                            )
                        nc.gpsimd.indirect_dma_start(
                            out=outs[(k, d0)][:],
                            out_offset=bass.IndirectOffsetOnAxis(
                                ap=idx_sb[
                                    :, G + colof[k] + j : G + colof[k] + j + 1
                                ],
                                axis=0,
                            ),
                            in_=so[:, :wt],
                            in_offset=None,
                            bounds_check=NTOK - 1,
                            oob_is_err=False,
                        )

            g_cur = gather_type(0)
            xts = {}
            for k in range(K):
                w1t = load_w1(k)
                b1t, b2t = load_biases(k)
                if k in xts:
                    xt = xts.pop(k)
                else:
                    xt = transpose_type(k, g_cur)
                ht = layer1(k, w1t, xt, b1t)
                if k + 1 < K:
                    g_cur = gather_type(k + 1)
                    if cfg["hoist_transposes"]:
                        xts[k + 1] = transpose_type(k + 1, g_cur)
                layer2(k, ht, b2t)

    nc.compile()
    return nc


def _routing(rx_shard, rt_shard, min_t):
    """Per-core routing tables.

    Returns (counts[k], token_lists[k], target_rows) where token_lists[k]
    holds flat token ids (s*BC + b) of head k in order, and target_rows[t]
    is the output row for flat token t.
    """
    rx_flat = rx_shard.reshape(-1)  # [NTOK], token t = s*BC + b
    ri = rt_shard - min_t[None, :]  # [S, BC]
    b_ids = np.broadcast_to(np.arange(BC, dtype=np.int64)[None, :], ri.shape)
    target = (ri.astype(np.int64) * BC + b_ids).reshape(-1)  # [NTOK]
    lists = [np.nonzero(rx_flat == k)[0] for k in range(K)]
    counts = [len(l) for l in lists]
    return counts, lists, target


def _pack_idx(caps, lists_per_core, targets_per_core):
    """Build the [128, 2G] int32 index tensor for one core."""
    G = sum(c // 128 for c in caps)
    arr = np.zeros((128, 2 * G), dtype=np.int32)
    col = 0
    for k in range(K):
        capk = caps[k]
        nj = capk // 128
        lst = lists_per_core[k]
        g = np.zeros(capk, dtype=np.int32)  # gather pad -> row 0 (safe)
        g[: len(lst)] = lst
        s = np.full(capk, OOB_SENTINEL, dtype=np.int32)  # scatter pad -> skipped
        s[: len(lst)] = targets_per_core[lst]
        for j in range(nj):
            arr[:, col + j] = g[j * 128 : (j + 1) * 128]
            arr[:, G + col + j] = s[j * 128 : (j + 1) * 128]
        col += nj
    return arr


def _prepare(inputs, cfg=None):
    """Shared host-side prep for kernel() and profiling runs."""
    x = np.ascontiguousarray(np.asarray(inputs["x"], dtype=np.float32))
    rx = np.asarray(inputs["readout_x"], dtype=np.int64)
    rt = np.asarray(inputs["readout_t"], dtype=np.int64)
    W1 = np.ascontiguousarray(np.asarray(inputs["W1"], dtype=np.float32))
    b1 = np.ascontiguousarray(np.asarray(inputs["b1"], dtype=np.float32))
    W2 = [
        np.ascontiguousarray(np.asarray(inputs[f"W2_{k}"], dtype=np.float32))
        for k in range(K)
    ]
    b2 = [
        np.ascontiguousarray(np.asarray(inputs[f"b2_{k}"], dtype=np.float32))
        for k in range(K)
    ]
    min_t = rt.min(axis=0)  # [B]

    per_core = []
    for c in range(NCORES):
        bsl = slice(c * BC, (c + 1) * BC)
        counts, lists, target = _routing(rx[:, bsl], rt[:, bsl], min_t[bsl])
        per_core.append((counts, lists, target))

    caps = tuple(
        max(128, int(-(-max(pc[0][k] for pc in per_core) // 128)) * 128)
        for k in range(K)
    )
    use_b1 = bool(np.any(b1))
    use_b2 = bool(np.any(np.concatenate([v.ravel() for v in b2])))

    key = (caps, use_b1, use_b2, tuple(sorted((cfg or {}).items())))
    if key not in _program_cache:
        _program_cache[key] = _build_program(caps, use_b1, use_b2, cfg)
    nc = _program_cache[key]

    in_maps = []
    for c in range(NCORES):
        counts, lists, target = per_core[c]
        x_shard = np.ascontiguousarray(
            x[:, c * BC : (c + 1) * BC, :]
        ).reshape(NTOK, C)
        m = {
            "x": x_shard,
            "w1": W1,
            "b1": b1,
            "idx": _pack_idx(caps, lists, target),
        }
        for k in range(K):
            m[f"w2_{k}"] = W2[k]
            m[f"b2_{k}"] = b2[k]
        in_maps.append(m)
    return nc, in_maps


def _run(inputs, cfg=None, **run_kwargs):
    nc, in_maps = _prepare(inputs, cfg)
    res = run_bass_kernel_spmd(
        nc, in_maps, core_ids=list(range(NCORES)), **run_kwargs
    )
    shards = []
    for c in range(NCORES):
        pieces = [res.results[c][f"out_{k}_{d0}"] for k, d0, _ in OUT_BLOCKS]
        shards.append(np.concatenate(pieces, axis=-1).reshape(S, BC, A))
    full = np.concatenate(shards, axis=1)
    return full, res


def kernel(**inputs) -> np.ndarray:
    full, _ = _run(inputs)
    return full
